# revision 1
# baseline (speedup 1.0000x reference)
"""Trainium2 Bass kernel for the distributed chunked hybrid contrastive loss.

Math (exactly equivalent to the reference up to fp error):
  loss = -(T/N)*W + [ sum_i lse_a_i + sum_j lse_b_j ] / (2N)
    W       = sum_c S_a[c].S_b[c] / max(count_c, 1)   (class feature sums)
    lse_a_i = log(sum_j exp(T fa_i.fb_j - T)) + T      (row sums)
    lse_b_j = log(sum_i exp(T fa_i.fb_j - T)) + T      (col sums; the
              logits matrix is evaluated once - the second direction
              reuses it via column sums, halving the exp work)

SPMD over 8 NeuronCores, data-parallel on the A-batch dim with B
replicated (per the sharding hint); column sums and class sums are
combined across cores with ReduceScatters; each core emits 3 partial
scalars which the host sums (the unshard step).

Schedule highlights (from perfetto-driven tuning):
  - ScalarE is the bottleneck: one exp per logits element at 1 elem/
    lane/cycle; exp reads PSUM [128, 2048] with fused accum_out row sums
  - fp16 features; PE matmuls LDWEIGHTS-deduped (consecutive identical
    stationary operands) => ~215ns per N=512 matmul
  - class-sum matmuls interleaved into the first main iterations
    (separate 4-bank PSUM pool beside a single-buffered phase A)
  - a dummy warmup collective absorbs the ~60us cold ncfw setup
  - per-block colsum partition-reduce (ones-matmul, col-group
    tile_position) deferred behind the next block's matmuls
"""

import numpy as np

import concourse.bass as bass
import concourse.mybir as mybir
from concourse import tile

N = 8192
D = 128
C_PAD = 1024
N_CORES = 8
BLK = 2048

# ---------------------------------------------------------------------------
# workarounds for this toolchain
# ---------------------------------------------------------------------------

def _install_patches():
    """(1) split >1-per-instruction sem waits (walrus CoreV3 allows one;
    2 on EventSemaphore); (2) patch the TileContext tail drain the same
    way. Idempotent."""
    import bass_rust
    from concourse.tile import TileContext, ScopedClock

    if getattr(TileContext, "_ccl_patched", False):
        return
    TileContext._ccl_patched = True

    def _drain_and_barrier(self, tick_clock, wait_clock):
        drain_inst = self.nc.sync.drain()
        wait_clock.add_sem_waits(drain_inst.ins,
                                 ScopedClock({None: tick_clock.global_clock}))
        si = drain_inst.ins.sync_info
        waits = list(si.on_wait or []) if si is not None else []
        if len(waits) > 1:
            drain_inst.ins.sync_info = bass_rust.SyncInfo(
                on_wait=waits[:1], on_update=si.on_update)
            rest = waits[1:]
            for i in range(len(rest)):
                d2 = self.nc.sync.drain()
                d2.ins.sync_info = bass_rust.SyncInfo(
                    on_wait=rest[i:i + 1], on_update=[])
        self.nc.all_engine_barrier()
        popped = self.nc._tile_sem_poison_stack.pop()
        assert popped is self._sem_poison
        self.nc.clear_and_free_semaphores(list(self.sems.allocated().values()))
        self.nc.all_engine_barrier()

    TileContext._drain_and_barrier = _drain_and_barrier


_UID = [0]


def _split_excess_waits(nc):
    import bass_rust
    for bb in nc.main_func.blocks:
        out = []
        changed = False
        for ins in bb.instructions:
            si = ins.sync_info
            waits = list(si.on_wait) if (si is not None and si.on_wait) else []
            cap = 2 if isinstance(ins, mybir.InstEventSemaphore) else 1
            if len(waits) > cap:
                keep, rest = waits[:cap], waits[cap:]
                for i in range(0, len(rest), 2):
                    _UID[0] += 1
                    ev = mybir.InstEventSemaphore(
                        name=f"waitsplit_{_UID[0]}", engine=ins.engine,
                        ins=[], outs=[],
                        sync_info=bass_rust.SyncInfo(on_wait=rest[i:i + 2],
                                                     on_update=[]))
                    nc.register_instruction(ev, overwrite=True)
                    out.append(ev)
                ins.sync_info = bass_rust.SyncInfo(
                    on_wait=keep, on_update=list(si.on_update or []))
                changed = True
            out.append(ins)
        if changed:
            bb.instructions = out
    return nc


def _dedup_ldweights(nc):
    """Drop consecutive redundant LDWEIGHTS on the PE stream (walrus here
    reloads the stationary operand before every matmul, serializing the
    array at ~390ns/MM instead of ~215ns)."""
    import bass_rust
    for bb in nc.main_func.blocks:
        out = []
        last_key = None
        for ins in bb.instructions:
            if ins.engine == mybir.EngineType.PE:
                if isinstance(ins, mybir.InstLdweights):
                    key = (str(ins.ins[0] if ins.ins else None),
                           str(ins.tile_position), str(ins.perf_mode),
                           str(ins.is_transpose))
                    if key == last_key:
                        si = ins.sync_info
                        waits = list(si.on_wait or []) if si else []
                        ups = list(si.on_update or []) if si else []
                        if waits or ups:
                            _UID[0] += 1
                            ev = mybir.InstEventSemaphore(
                                name=f"lddedup_{_UID[0]}", engine=ins.engine,
                                ins=[], outs=[],
                                sync_info=bass_rust.SyncInfo(
                                    on_wait=waits, on_update=ups))
                            nc.register_instruction(ev, overwrite=True)
                            out.append(ev)
                        continue
                    last_key = key
            out.append(ins)
        bb.instructions = out
    return nc


# ---------------------------------------------------------------------------
# device program
# ---------------------------------------------------------------------------

import concourse.bass as bass

F16 = mybir.dt.float16
BF16 = mybir.dt.bfloat16
F32 = mybir.dt.float32
I32 = mybir.dt.int32
AX = mybir.AxisListType
ALU = mybir.AluOpType
ACTF = mybir.ActivationFunctionType
LN2 = 0.6931471805599453
PA = 5          # main iterations sharing PSUM with the class phase
CLS_PER_IT = 7  # class matmuls interleaved after each early iteration


def emit_sum_log(nc, pool, x_ap, p, v, tag):
    """sum_free log(x) for positive normal f32 x via exponent extraction."""
    xi = x_ap.bitcast(I32)
    e_i = pool.tile([p, v], I32, tag=f"{tag}_ei")
    nc.vector.tensor_scalar(out=e_i[:, :], in0=xi, scalar1=23, scalar2=None,
                            op0=ALU.logical_shift_right)
    m_i = pool.tile([p, v], I32, tag=f"{tag}_mi")
    nc.vector.tensor_scalar(out=m_i[:, :], in0=xi, scalar1=0x007FFFFF,
                            scalar2=0x3F800000, op0=ALU.bitwise_and,
                            op1=ALU.bitwise_or)
    e_f = pool.tile([p, v], F32, tag=f"{tag}_ef")
    nc.vector.tensor_copy(e_f[:, :], e_i[:, :])
    e_sum = pool.tile([p, 1], F32, tag=f"{tag}_es")
    nc.vector.reduce_sum(out=e_sum[:, :], in_=e_f[:, :], axis=AX.X)
    lnm = pool.tile([p, v], F32, tag=f"{tag}_lnm")
    lm1 = pool.tile([p, 1], F32, tag=f"{tag}_lm1")
    nc.scalar.activation(out=lnm[:, :], in_=m_i[:, :].bitcast(F32),
                         func=ACTF.Ln, accum_out=lm1[:, :])
    nc.vector.tensor_scalar(out=e_sum[:, :], in0=e_sum[:, :],
                            scalar1=float(127 * v), scalar2=LN2,
                            op0=ALU.subtract, op1=ALU.mult)
    nc.vector.tensor_tensor(out=lm1[:, :], in0=lm1[:, :], in1=e_sum[:, :],
                            op=ALU.add)
    return lm1


def build(T_val: float, n: int = 8192, d: int = 128, c_pad: int = 1024,
          n_cores: int = 8, blk: int = 2048):
    local = n // n_cores
    rt = local // 128
    nblk = n // blk
    csch = blk // 512
    assert d == 128 and local % 128 == 0 and blk == 2048
    rg = [list(range(n_cores))]
    nrow = 128 // n_cores
    cs_slice = blk // n_cores

    nc = bass.Bass(num_devices=n_cores)

    # packed per-core f16 inputs: [faT | fa_rows | fb_rows | t_loc]
    pk_w = 3 * local + rt
    pk = nc.declare_dram_parameter("pk", [128, pk_w], F16, isOutput=False)
    fbT = nc.declare_dram_parameter("fbT", [128, n], F16, isOutput=False)
    recip_cnt = nc.declare_dram_parameter("recip_cnt", [nrow, c_pad], F32, isOutput=False)
    out_part = nc.declare_dram_parameter("out_part", [1, 4], F32, isOutput=True)

    warm_in = nc.dram_tensor("warm_in", [64], F32)
    warm_out = nc.dram_tensor("warm_out", [8], F32)
    s_part = nc.dram_tensor("s_part", [128, 2 * c_pad], BF16)
    s_mine = nc.dram_tensor("s_mine", [nrow, 2 * c_pad], BF16)
    cs_part = nc.dram_tensor("cs_part", [n], F32)
    cs_mine = nc.dram_tensor("cs_mine", [local], F32)

    with tile.TileContext(nc) as tc:
        with (
            tc.tile_pool(name="const", bufs=1) as cpool,
            tc.tile_pool(name="work", bufs=1) as wpool,
            tc.tile_pool(name="exps", bufs=3) as epool,
        ):
            # warmup collective (no data deps; ~60us cold setup runs async)
            nc.gpsimd.collective_compute(
                "ReduceScatter", ALU.add, replica_groups=rg,
                ins=[warm_in[:].opt()], outs=[warm_out[:].opt()])

            # ---- DMAs: fbT chunk 0 + packed inputs first ----
            fbT_sb = cpool.tile([128, n], F16, tag="fbT")
            nc.sync.dma_start(fbT_sb[:, 0:512], fbT[:, 0:512])
            pk_sb = cpool.tile([128, pk_w], F16, tag="pk")
            nc.sync.dma_start(pk_sb[:, 0:local], pk[:, 0:local])
            nc.sync.dma_start(fbT_sb[:, 512:2048], fbT[:, 512:2048])
            nc.sync.dma_start(pk_sb[:, local:], pk[:, local:])
            faT_sb = pk_sb[:, 0:local]
            fa_r_sb = pk_sb[:, local:2 * local]
            fb_r_sb = pk_sb[:, 2 * local:3 * local]
            t_sb = pk_sb[:, 3 * local:3 * local + rt]
            recip_sb = cpool.tile([nrow, c_pad], F32, tag="recip")
            nc.sync.dma_start(recip_sb[:, :], recip_cnt[:, :])
            for q in range(1, 4):
                s = q * (n // 4)
                e = (q + 1) * (n // 4)
                nc.sync.dma_start(fbT_sb[:, s:e], fbT[:, s:e])

            ones_bf = cpool.tile([128, 32], BF16, tag="ones_bf")
            nc.gpsimd.memset(ones_bf[:, :], 1.0)
            ones_f32 = cpool.tile([128, 1], F32, tag="ones_f32")
            nc.gpsimd.memset(ones_f32[:, :], 1.0)
            neg_t = cpool.tile([128, 1], F32, tag="neg_t")
            nc.gpsimd.memset(neg_t[:, :], -T_val)

            rparts = wpool.tile([128, rt * nblk + PA], F32, tag="rparts")
            out_sb = wpool.tile([1, 4], F32, tag="out_sb")
            iota_i = wpool.tile([128, c_pad], I32, tag="iota_i")
            nc.gpsimd.iota(iota_i[:, :], pattern=[[1, c_pad]], base=0,
                           channel_multiplier=0)
            iota_f = wpool.tile([128, c_pad], F16, tag="iota_f")
            nc.vector.tensor_copy(iota_f[:, :], iota_i[:, :])
            t_f32 = wpool.tile([128, rt], F32, tag="t_f32")
            nc.vector.tensor_copy(t_f32[:, :], t_sb)

            # one-hot tiles (DVE tensor_scalar, 4x mode)
            oh_g = []
            for g in range(rt):
                ohg = wpool.tile([128, c_pad], F16, tag=f"oh{g}")
                nc.vector.tensor_scalar(
                    out=ohg[:, :], in0=iota_f[:, :],
                    scalar1=t_f32[:, g:g + 1], scalar2=None,
                    op0=ALU.is_equal)
                oh_g.append(ohg)

            # class + phase-A PSUM pools side by side (4 + 4 banks)
            clstack = tc.tile_pool(name="clsps", bufs=1, space="PSUM")
            clspool = clstack.__enter__()
            mstackA = tc.tile_pool(name="mmpsA", bufs=2, space="PSUM")
            mpoolA = mstackA.__enter__()
            sa_ps = clspool.tile([128, c_pad], F32, tag="sa")
            sb_ps = clspool.tile([128, c_pad], F32, tag="sb")

            cls_jobs = []
            for g in range(rt):
                for lhs, ps in ((fa_r_sb, sa_ps), (fb_r_sb, sb_ps)):
                    for h in range(c_pad // 512):
                        cls_jobs.append((g, lhs, ps, h))

            def emit_cls(njobs):
                while njobs > 0 and cls_jobs:
                    g, lhs, ps, h = cls_jobs.pop(0)
                    nc.tensor.matmul(
                        ps[:, h * 512:(h + 1) * 512],
                        lhsT=lhs[:, g * 128:(g + 1) * 128],
                        rhs=oh_g[g][:, h * 512:(h + 1) * 512],
                        start=(g == 0), stop=(g == rt - 1))
                    njobs -= 1

            last_exp = None

            e_accs = {}

            def iteration(b, r, mpool, half=None, pa_idx=0):
                nonlocal last_exp
                if r == 0 and (half is None or half == 0):
                    e_accs[b] = epool.tile([128, blk], BF16, tag="eacc",
                                           name="eacc")
                e_acc = e_accs[b]
                w = blk if half is None else blk // 2
                c0 = b * blk + (0 if not half else w)
                mtag = "mm" if half is None else "mmA"
                mm = mpool.tile([128, w], F32, tag=mtag, name="mm")
                lhs = faT_sb[:, r * 128:(r + 1) * 128]
                for h in range(w // 512):
                    nc.tensor.matmul(
                        mm[:, h * 512:(h + 1) * 512], lhsT=lhs,
                        rhs=fbT_sb[:, c0 + h * 512: c0 + (h + 1) * 512],
                        start=True, stop=True)
                expt = epool.tile([128, w], BF16,
                                  tag="exp" if half is None else "expA",
                                  name="expt")
                acc_idx = (b * rt + r) if (half is None or half == 0) \
                    else (rt * nblk + pa_idx)
                last_exp = nc.scalar.activation(
                    out=expt[:, :], in_=mm[:, :], func=ACTF.Exp,
                    bias=neg_t[:, :], scale=T_val,
                    accum_out=rparts[:, acc_idx:acc_idx + 1])
                ea = e_acc[:, c0 - b * blk: c0 - b * blk + w]
                if r == 0:
                    nc.vector.tensor_copy(ea, expt[:, :])
                else:
                    nc.vector.tensor_tensor(out=ea, in0=ea, in1=expt[:, :],
                                            op=ALU.add)

            def emit_cs(b, mpool):
                # colsum partition-reduce + drain for a finished block;
                # deferred so it sits behind the next block's first matmuls
                # in the PE queue.
                e_acc = e_accs.pop(b)
                cs = mpool.tile([128, blk], F32, tag="mm")
                for h in range(csch):
                    nc.tensor.matmul(cs[32 * h:32 * (h + 1), 0:512],
                                     lhsT=ones_bf[:, :],
                                     rhs=e_acc[:, h * 512:(h + 1) * 512],
                                     start=True, stop=True,
                                     tile_position=(0, 32 * h))
                cls_sb = wpool.tile([128, 512], F32, tag=f"cls_sb{b % 2}")
                nc.vector.tensor_copy(cls_sb[:, :], cs[:, 0:512])
                src_rows = cls_sb[:, :].rearrange(
                    "(g p) f -> g p f", g=4)[:, 0, :]
                nc.sync.dma_start(cs_part[b * blk:(b + 1) * blk], src_rows)

            seq = [(b, r) for b in range(nblk) for r in range(rt)]
            mpool = mpoolA
            for j in range(PA):
                b, r = seq[j]
                for half in (0, 1):
                    iteration(b, r, mpoolA, half=half, pa_idx=j)
                    emit_cls(4)
            for i, (b, r) in enumerate(seq):
                if i < PA:
                    continue
                if i == PA:
                    emit_cls(len(cls_jobs))  # flush any remainder
                    # drain class sums, kick off their ReduceScatter
                    sa_sb = wpool.tile([128, c_pad], BF16, tag="sa_sb")
                    sb_sb = wpool.tile([128, c_pad], BF16, tag="sb_sb")
                    nc.vector.tensor_copy(sa_sb[:, :], sa_ps[:, :])
                    nc.vector.tensor_copy(sb_sb[:, :], sb_ps[:, :])
                    nc.sync.dma_start(s_part[:, 0:c_pad], sa_sb[:, :])
                    nc.sync.dma_start(s_part[:, c_pad:2 * c_pad], sb_sb[:, :])
                    nc.gpsimd.collective_compute(
                        "ReduceScatter", ALU.add, replica_groups=rg,
                        ins=[s_part[:, :].opt()], outs=[s_mine[:, :].opt()])
                    mstackA.__exit__(None, None, None)
                    clstack.__exit__(None, None, None)
                    mstackB = tc.tile_pool(name="mmpsB", bufs=2, space="PSUM")
                    mpool = mstackB.__enter__()
                iteration(b, r, mpool)
                if r == 1 and b > 0:
                    emit_cs(b - 1, mpool)
            emit_cs(nblk - 1, mpool)
            mstackB.__exit__(None, None, None)

            # ---- finalize ----
            fstack = tc.tile_pool(name="finps", bufs=2, space="PSUM")
            fpool = fstack.__enter__()

            rsum = wpool.tile([128, rt], F32, tag="rsum")
            nc.vector.tensor_reduce(
                out=rsum[:, :],
                in_=rparts[:, 0:rt * nblk].rearrange("p (b r) -> p r b",
                                                     b=nblk),
                axis=AX.X, op=ALU.add)
            nc.vector.tensor_tensor(
                out=rsum[:, 0:PA], in0=rsum[:, 0:PA],
                in1=rparts[:, rt * nblk:rt * nblk + PA], op=ALU.add)
            lr1 = emit_sum_log(nc, wpool, rsum[:, :], 128, rt, "lr")
            lr_ps = fpool.tile([1, 1], F32, tag="cs")
            nc.tensor.matmul(lr_ps[:, :], lhsT=ones_f32[:, :], rhs=lr1[:, :],
                             start=True, stop=True)
            nc.scalar.copy(out_sb[0:1, 0:1], lr_ps[0:1, 0:1])

            nc.gpsimd.collective_compute(
                "ReduceScatter", ALU.add, replica_groups=rg,
                ins=[cs_part[:].opt()], outs=[cs_mine[:].opt()])
            ncol = local // 128
            csg = wpool.tile([128, ncol], F32, tag="csg")
            nc.sync.dma_start(csg[:, :], cs_mine[:])
            lc1 = emit_sum_log(nc, wpool, csg[:, :], 128, ncol, "lc")
            lc_ps = fpool.tile([1, 1], F32, tag="cs")
            nc.tensor.matmul(lc_ps[:, :], lhsT=ones_f32[:, :], rhs=lc1[:, :],
                             start=True, stop=True)
            nc.scalar.copy(out_sb[0:1, 1:2], lc_ps[0:1, 0:1])

            sm_sb = wpool.tile([nrow, 2 * c_pad], BF16, tag="sm_sb")
            sm_dma = nc.sync.dma_start(sm_sb[:, :], s_mine[:, :])
            tile.add_dep_helper(sm_dma.ins, last_exp.ins, sync=False,
                                reason="pin W-chain after main loop")
            prod = wpool.tile([nrow, c_pad], F32, tag="prod")
            nc.vector.tensor_tensor(out=prod[:, :], in0=sm_sb[:, 0:c_pad],
                                    in1=sm_sb[:, c_pad:2 * c_pad], op=ALU.mult)
            nc.vector.tensor_tensor(out=prod[:, :], in0=prod[:, :],
                                    in1=recip_sb[:, :], op=ALU.mult)
            wred = wpool.tile([nrow, 1], F32, tag="wred")
            nc.vector.tensor_reduce(out=wred[:, :], in_=prod[:, :], axis=AX.X,
                                    op=ALU.add)
            w_ps = fpool.tile([1, 1], F32, tag="cs")
            nc.tensor.matmul(w_ps[:, :], lhsT=ones_f32[0:nrow, :], rhs=wred[:, :],
                             start=True, stop=True)
            nc.scalar.copy(out_sb[0:1, 2:3], w_ps[0:1, 0:1])
            nc.gpsimd.memset(out_sb[0:1, 3:4], 0.0)

            nc.sync.dma_start(out_part[:, :], out_sb[:, :])
            fstack.__exit__(None, None, None)

    return nc

# ---------------------------------------------------------------------------
# host wrapper
# ---------------------------------------------------------------------------

_PROGRAM_CACHE = {}


def _get_program(t_val):
    key = float(t_val)
    if key not in _PROGRAM_CACHE:
        _install_patches()
        nc = build(key, n=N, d=D, c_pad=C_PAD, n_cores=N_CORES, blk=BLK)
        _split_excess_waits(nc)
        _dedup_ldweights(nc)
        _PROGRAM_CACHE[key] = nc
    return _PROGRAM_CACHE[key]


def _prep_in_maps(fa, fb, targets):
    local = N // N_CORES
    rt = local // 128
    tgt = targets.astype(np.int64)
    counts = np.bincount(tgt, minlength=C_PAD).astype(np.float32)
    recip = (1.0 / np.maximum(counts, 1.0)).astype(np.float32)
    nrow = 128 // N_CORES
    recip_rep = np.ascontiguousarray(np.broadcast_to(recip, (nrow, C_PAD)))
    fbT = np.ascontiguousarray(fb.T.astype(np.float16))
    in_maps = []
    for k in range(N_CORES):
        sl = slice(k * local, (k + 1) * local)
        fa_loc = fa[sl]
        fb_loc = fb[sl]
        pk = np.concatenate([
            fa_loc.T.astype(np.float16),
            fa_loc.reshape(rt, 128, D).transpose(1, 0, 2).reshape(128, rt * D)
            .astype(np.float16),
            fb_loc.reshape(rt, 128, D).transpose(1, 0, 2).reshape(128, rt * D)
            .astype(np.float16),
            tgt[sl].reshape(rt, 128).T.astype(np.float16),
        ], axis=1)
        in_maps.append({
            "pk": np.ascontiguousarray(pk),
            "fbT": fbT,
            "recip_cnt": recip_rep,
        })
    return in_maps


def kernel(T, local_features_a, local_features_b, global_targets,
           training=None, **_unused):
    """Full (unsharded) inputs in; full scalar loss out. Shards across the
    8 NeuronCores internally, runs the Bass kernel SPMD, and combines the
    per-core partial scalars (the unshard step)."""
    from concourse.bass_utils import run_bass_kernel_spmd

    t_val = float(np.asarray(T).reshape(-1)[0])
    fa = np.asarray(local_features_a, dtype=np.float32)
    fb = np.asarray(local_features_b, dtype=np.float32)
    tg = np.asarray(global_targets)
    assert fa.shape == (N, D) and fb.shape == (N, D)

    nc = _get_program(t_val)
    in_maps = _prep_in_maps(fa, fb, tg)
    res = run_bass_kernel_spmd(nc, in_maps, list(range(N_CORES)))

    o = np.stack([np.asarray(r["out_part"][0], dtype=np.float64)
                  for r in res.results])
    sum_log_rows = o[:, 0].sum()
    sum_log_cols = o[:, 1].sum()
    w = o[:, 2].sum()
    loss = (sum_log_rows + N * t_val + sum_log_cols + N * t_val) \
        / (2.0 * N) - (t_val / N) * w
    return np.float32(loss)

